# revision 25
# baseline (speedup 1.0000x reference)
"""AffineEdgeAttention Trainium2 kernel (bf16-streamed, PE-centric).

out[b, i, j] = head[b, i] . w_h + dep[b, j] . w_d + edge_b

Sharding: data-parallel over batch; 16 batches / 8 cores = 2 per core.

The 2e-2 tolerance admits bf16 streaming (measured end-to-end rel err
~5e-3), which halves HBM traffic vs f32: per core 6 MiB of loads +
4 MiB of stores = 10.4 MiB ~= 25 us at the ~430 GB/s the 16 SDMA
engines sustain on contiguous >=2 KB/partition descriptors.

Layout/engine plan per core:
  - host pre-transposes head/dep to [d, row] chunk-major form so every
    load is contiguous 4 KB-per-partition descriptors (line rate) and
    the PE can contract over d on the partition axis. Loads are split
    per chunk-pair so matmuls chase the arriving FIFO stream.
  - w/bias ride in as a tiny [128, 16] f32 load dispatched FIRST on the
    sync ring: FIFO order drains its 64 B descriptors before the big
    loads start (on the other ring they round-robin one per 456 ns turn
    and land ~17 us in).
  - PE p-state ramps over ~3 us, and the ACT function table loads
    lazily (~1.5 us): both are warmed with dummy ops during the load
    latency so real matmuls run at full rate.
  - head pass: 12 matmuls with lhsT = w_h chunk [128, 1] accumulate the
    s_h row [1, S]; 8 tiny K=1 bf16 matmuls transpose it into a
    per-partition column [128, 8]. Head goes first: its chain to the
    adds is longer than dep's.
  - dep pass: 12 bf16 matmuls with lhsT = w_d chunk column broadcast
    (free-stride 0) accumulate s_d directly *broadcast* across all 128
    partitions of PSUM [128, S]; the +bias PSUM->SBUF copy is split
    ACT-half/DVE-half so it is off the critical path (same for s_h row).
  - outputs: 16 bf16 tensor_scalar adds on DVE (4x perf mode), stored
    as [128, 2, 1024] tiles, all on the sync ring (ACT dispatching them
    would block its later copies behind the store semaphore waits).
"""

import sys

import numpy as np

for _p in ("/opt/trn_rl_repo", "/root/.axon_site/_ro/trn_rl_repo"):
    if _p not in sys.path:
        sys.path.insert(0, _p)

import ml_dtypes

import concourse.bacc as bacc
import concourse.bass as bass
import concourse.tile as tile
from concourse import mybir
from concourse.bass_utils import run_bass_kernel_spmd

B, S, D = 16, 1024, 768
N_CORES = 8
BPC = B // N_CORES  # batches per core
P = 128
DC = D // P  # 6 d-chunks
RC = S // P  # 8 row chunks
NPAIR = RC // 2
HALF = S // 2  # psum bank boundary: 512 f32
N_WARM = 12  # PE p-state warmup matmuls (fill the load-latency window)

F32 = mybir.dt.float32
BF16 = mybir.dt.bfloat16
F8 = mybir.dt.float8e4
NP_BF16 = ml_dtypes.bfloat16
NP_F8 = ml_dtypes.float8_e4m3


def build_program() -> bass.Bass:
    nc = bacc.Bacc("TRN2", target_bir_lowering=False, debug=False)
    head = nc.dram_tensor("head", [BPC, P, DC, S], BF16, kind="ExternalInput").ap()
    dep = nc.dram_tensor("dep", [BPC, P, DC, S], F8, kind="ExternalInput").ap()
    wb = nc.dram_tensor("wb", [P, 16], F32, kind="ExternalInput").ap()
    out = nc.dram_tensor("out", [BPC, NPAIR, P, 2, S], BF16, kind="ExternalOutput").ap()

    with tile.TileContext(nc) as tc:
        with (
            tc.tile_pool(name="singles", bufs=1) as singles,
            tc.tile_pool(name="loads", bufs=BPC) as loads,
            tc.tile_pool(name="bcast", bufs=BPC) as bcast,
            tc.tile_pool(name="svec", bufs=BPC) as svec,
            tc.tile_pool(name="outs", bufs=BPC * NPAIR) as outs,
            tc.tile_pool(name="ps_wrm", bufs=1, space="PSUM") as psum_warm,
            tc.tile_pool(name="ps_sdb", bufs=BPC, space="PSUM") as psum_sdb,
            tc.tile_pool(name="ps_shr", bufs=1, space="PSUM") as psum_shr,
            tc.tile_pool(name="ps_shc", bufs=1, space="PSUM") as psum_shc,
        ):
            # ---- sync ring: first dispatch is real data (head0 pair0),
            # w/bias rides second (FIFO -> still lands by ~9 us), then the
            # rest of the stream split per chunk-pair. Head leads each
            # batch: its chain to the output adds is the long one.
            in_tiles = []
            for b in range(BPC):
                ht_ = loads.tile([P, DC, S], BF16, tag="head")
                dt_ = loads.tile([P, DC, S], F8, tag="dep")
                in_tiles.append((ht_, dt_))
            # w/bias on the (empty) scalar ring: drains before Q1 traffic
            # starts, so its completion sem fires early.
            wbt = singles.tile([P, 16], F32)
            nc.scalar.dma_start(out=wbt, in_=wb)
            # first head pair as two single-chunk DMAs -> earliest first sem
            nc.sync.dma_start(out=in_tiles[0][0][:, 0:1], in_=head[0, :, 0:1])
            nc.sync.dma_start(out=in_tiles[0][0][:, 1:2], in_=head[0, :, 1:2])
            for b in range(BPC):
                ht_, dt_ = in_tiles[b]
                for pr in range(DC // 2):
                    if b == 0 and pr == 0:
                        continue
                    nc.sync.dma_start(
                        out=ht_[:, 2 * pr : 2 * pr + 2], in_=head[b, :, 2 * pr : 2 * pr + 2]
                    )
                for pr in range(DC // 2):
                    if b == BPC - 1 and pr == DC // 2 - 1:
                        # last dep pair as two chunks -> earlier final sem
                        nc.sync.dma_start(
                            out=dt_[:, 2 * pr : 2 * pr + 1], in_=dep[b, :, 2 * pr : 2 * pr + 1]
                        )
                        nc.sync.dma_start(
                            out=dt_[:, 2 * pr + 1 : 2 * pr + 2],
                            in_=dep[b, :, 2 * pr + 1 : 2 * pr + 2],
                        )
                    else:
                        nc.sync.dma_start(
                            out=dt_[:, 2 * pr : 2 * pr + 2], in_=dep[b, :, 2 * pr : 2 * pr + 2]
                        )

            # ---- engine warmup during the load latency ----
            warm_sb = singles.tile([P, 256], BF16)
            nc.vector.memset(warm_sb, 1.0)
            ones11b = singles.tile([1, 1], BF16)
            nc.vector.memset(ones11b, 1.0)
            warm_act = singles.tile([1, 1], F32)
            nc.scalar.copy(out=warm_act, in_=ones11b)  # triggers ACT table load
            ps_warm = psum_warm.tile([P, 256], F32)
            for i in range(N_WARM):
                nc.tensor.matmul(
                    ps_warm,
                    lhsT=warm_sb[:, :1].broadcast_to((P, P)),
                    rhs=warm_sb,
                    start=True,
                    stop=True,
                )
            wct = singles.tile([P, 2 * DC], BF16)
            nc.vector.tensor_copy(wct, wbt[:, : 2 * DC])
            bt = wbt[:, 2 * DC : 2 * DC + 1]  # f32 bias column, used as AP

            for b in range(BPC):
                ht_, dt_ = in_tiles[b]

                # s_h row [1, S] (head leads the stream)
                ps_shr = psum_shr.tile([1, S], F32, tag="shr")
                for h in range(2):
                    for dc in range(DC):
                        nc.tensor.matmul(
                            ps_shr[:, h * HALF : (h + 1) * HALF],
                            lhsT=wct[:, DC + dc : DC + dc + 1],
                            rhs=ht_[:, dc, h * HALF : (h + 1) * HALF],
                            start=(dc == 0),
                            stop=(dc == DC - 1),
                        )
                shr_sb = svec.tile([1, S], BF16, tag="shr_sb")
                nc.scalar.copy(out=shr_sb, in_=ps_shr)

                # s_d broadcast into PSUM [128, S] (accumulate over d-chunks)
                ps_sdb = psum_sdb.tile([P, S], F32, tag="sdb")
                for h in range(2):
                    for dc in range(DC):
                        nc.tensor.matmul(
                            ps_sdb[:, h * HALF : (h + 1) * HALF],
                            lhsT=wct[:, dc : dc + 1].broadcast_to((P, P)),
                            rhs=dt_[:, dc, h * HALF : (h + 1) * HALF],
                            start=(dc == 0),
                            stop=(dc == DC - 1),
                        )
                # ACT takes the h0 half (its matmul group retires ~1 us before
                # h1's, so the copy overlaps the h1 matmuls); DVE takes the
                # late h1 half so the adds' last dependency is on-engine.
                sdb_sb = bcast.tile([P, S], BF16, tag="sdb_sb")
                nc.scalar.add(out=sdb_sb[:, :HALF], in_=ps_sdb[:, :HALF], add=bt)
                nc.vector.tensor_scalar_add(sdb_sb[:, HALF:], ps_sdb[:, HALF:], bt)

                # transpose s_h row into per-partition columns [128, 8]
                ps_shc = psum_shc.tile([P, RC], F32, tag="shc")
                for c in range(RC):
                    nc.tensor.matmul(
                        ps_shc[:, c : c + 1],
                        lhsT=shr_sb[:, c * P : (c + 1) * P],
                        rhs=ones11b,
                        start=True,
                        stop=True,
                    )
                shc = svec.tile([P, RC], F32, tag="shc_sb")
                nc.vector.tensor_copy(shc, ps_shc)

                # outputs: chunk c rows get sdb_sb + s_h[c*128+p]. b0 stores
                # all ride the sync ring (ACT dispatching them would block
                # its b1 copies behind the store waits); b1 pairs split
                # across both rings to parallelize the final dispatches.
                last = b == BPC - 1
                for t in range(NPAIR):
                    ot = outs.tile([P, 2, S], BF16, tag="ot")
                    for i in range(2):
                        c = 2 * t + i
                        if last and c == 0:
                            # one add on the otherwise-idle ACT shortens the
                            # DVE queue on the kernel's critical tail
                            nc.scalar.add(
                                out=ot[:, i, :], in_=sdb_sb, add=shc[:, c : c + 1]
                            )
                        else:
                            nc.vector.tensor_scalar_add(
                                ot[:, i, :], sdb_sb, shc[:, c : c + 1]
                            )
                    if last and t == NPAIR - 1:
                        # final pair as two chunk stores on both rings: the
                        # very last store is small and dispatches instantly
                        nc.sync.dma_start(out=out[b, t, :, 0], in_=ot[:, 0])
                        nc.scalar.dma_start(out=out[b, t, :, 1], in_=ot[:, 1])
                    else:
                        eng = nc.scalar if (last and t % 2 == 0) else nc.sync
                        eng.dma_start(out=out[b, t], in_=ot)
    nc.compile()
    return nc


def _prep_input(x: np.ndarray, dtype) -> np.ndarray:
    """[B, S, D] f32 -> [B, P, DC, S] with [b, p, c, j] = x[b, j, c*P+p]."""
    xt = x.astype(dtype).transpose(0, 2, 1)  # [B, D, S] view
    xt = xt.reshape(B, DC, P, S)  # forces the copy
    return xt.swapaxes(1, 2)  # [B, P, DC, S] view


def kernel(head, dep, edge_W, edge_b, _trace=False):
    nc = build_program()

    head_t = _prep_input(head, NP_BF16)
    dep_t = _prep_input(dep, NP_F8)
    # wb[p, i] = w_d[i*128+p] (i<6) | w_h[(i-6)*128+p] (6<=i<12) | b | pad
    wb = np.zeros((P, 16), dtype=np.float32)
    wb[:, :DC] = edge_W[0, D:].reshape(DC, P).T
    wb[:, DC : 2 * DC] = edge_W[0, :D].reshape(DC, P).T
    wb[:, 2 * DC] = edge_b[0]

    in_maps = []
    for k in range(N_CORES):
        in_maps.append(
            {
                "head": np.ascontiguousarray(head_t[k * BPC : (k + 1) * BPC]),
                "dep": np.ascontiguousarray(dep_t[k * BPC : (k + 1) * BPC]),
                "wb": wb,
            }
        )
    res = run_bass_kernel_spmd(nc, in_maps, core_ids=list(range(N_CORES)), trace=_trace)
    raw = np.concatenate([r["out"] for r in res.results], axis=0)  # [B,4,P,2,S] bf16
    out = (
        raw.transpose(0, 1, 3, 2, 4).reshape(B, S, S).astype(np.float32)
    )
    if _trace:
        return out, res
    return out


if __name__ == "__main__":
    rng = np.random.default_rng(0)
    head = rng.standard_normal((B, S, D), dtype=np.float32)
    dep = rng.standard_normal((B, S, D), dtype=np.float32)
    edge_W = rng.standard_normal((1, 2 * D), dtype=np.float32)
    edge_b = rng.standard_normal((1,), dtype=np.float32)
    out = kernel(head, dep, edge_W, edge_b)
    ref = (
        head @ edge_W[0, :D]
    )[:, :, None] + (dep @ edge_W[0, D:])[:, None, :] + edge_b[0]
    err = np.abs(out - ref).max() / np.abs(ref).max()
    print("max rel err:", err)


# revision 26
# speedup vs baseline: 1.1220x; 1.1220x over previous
"""AffineEdgeAttention Trainium2 kernel (bf16-streamed, PE-centric).

out[b, i, j] = head[b, i] . w_h + dep[b, j] . w_d + edge_b

Sharding: data-parallel over batch; 16 batches / 8 cores = 2 per core.

The 2e-2 tolerance admits bf16 streaming (measured end-to-end rel err
~5e-3), which halves HBM traffic vs f32: per core 6 MiB of loads +
4 MiB of stores = 10.4 MiB ~= 25 us at the ~430 GB/s the 16 SDMA
engines sustain on contiguous >=2 KB/partition descriptors.

Layout/engine plan per core:
  - host pre-transposes head/dep to [d, row] chunk-major form so every
    load is contiguous 4 KB-per-partition descriptors (line rate) and
    the PE can contract over d on the partition axis. Loads are split
    per chunk-pair so matmuls chase the arriving FIFO stream.
  - w/bias ride in as a tiny [128, 16] f32 load dispatched FIRST on the
    sync ring: FIFO order drains its 64 B descriptors before the big
    loads start (on the other ring they round-robin one per 456 ns turn
    and land ~17 us in).
  - PE p-state ramps over ~3 us, and the ACT function table loads
    lazily (~1.5 us): both are warmed with dummy ops during the load
    latency so real matmuls run at full rate.
  - head pass: 12 matmuls with lhsT = w_h chunk [128, 1] accumulate the
    s_h row [1, S]; 8 tiny K=1 bf16 matmuls transpose it into a
    per-partition column [128, 8]. Head goes first: its chain to the
    adds is longer than dep's.
  - dep pass: 12 bf16 matmuls with lhsT = w_d chunk column broadcast
    (free-stride 0) accumulate s_d directly *broadcast* across all 128
    partitions of PSUM [128, S]; the +bias PSUM->SBUF copy is split
    ACT-half/DVE-half so it is off the critical path (same for s_h row).
  - outputs: 16 bf16 tensor_scalar adds on DVE (4x perf mode), stored
    as [128, 2, 1024] tiles, all on the sync ring (ACT dispatching them
    would block its later copies behind the store semaphore waits).
"""

import sys

import numpy as np

for _p in ("/opt/trn_rl_repo", "/root/.axon_site/_ro/trn_rl_repo"):
    if _p not in sys.path:
        sys.path.insert(0, _p)

import ml_dtypes

import concourse.bacc as bacc
import concourse.bass as bass
import concourse.tile as tile
from concourse import mybir
from concourse.bass_utils import run_bass_kernel_spmd

B, S, D = 16, 1024, 768
N_CORES = 8
BPC = B // N_CORES  # batches per core
P = 128
DC = D // P  # 6 d-chunks
RC = S // P  # 8 row chunks
NPAIR = RC // 2
HALF = S // 2  # psum bank boundary: 512 f32
N_WARM = 12  # PE p-state warmup matmuls (fill the load-latency window)

F32 = mybir.dt.float32
BF16 = mybir.dt.bfloat16
F8 = mybir.dt.float8e4
NP_BF16 = ml_dtypes.bfloat16
NP_F8 = ml_dtypes.float8_e4m3


def build_program() -> bass.Bass:
    nc = bacc.Bacc("TRN2", target_bir_lowering=False, debug=False)
    head = nc.dram_tensor("head", [BPC, P, DC, S], BF16, kind="ExternalInput").ap()
    dep = nc.dram_tensor("dep", [BPC, P, DC, S], F8, kind="ExternalInput").ap()
    wb = nc.dram_tensor("wb", [P, 16], F32, kind="ExternalInput").ap()
    out = nc.dram_tensor("out", [BPC, NPAIR, P, 2, S], BF16, kind="ExternalOutput").ap()

    with tile.TileContext(nc) as tc:
        with (
            tc.tile_pool(name="singles", bufs=1) as singles,
            tc.tile_pool(name="loads", bufs=BPC) as loads,
            tc.tile_pool(name="bcast", bufs=BPC) as bcast,
            tc.tile_pool(name="svec", bufs=BPC) as svec,
            tc.tile_pool(name="outs", bufs=BPC * NPAIR) as outs,
            tc.tile_pool(name="ps_wrm", bufs=1, space="PSUM") as psum_warm,
            tc.tile_pool(name="ps_sdb", bufs=BPC, space="PSUM") as psum_sdb,
            tc.tile_pool(name="ps_shr", bufs=1, space="PSUM") as psum_shr,
            tc.tile_pool(name="ps_shc", bufs=1, space="PSUM") as psum_shc,
        ):
            # ---- sync ring: first dispatch is real data (head0 pair0),
            # w/bias rides second (FIFO -> still lands by ~9 us), then the
            # rest of the stream split per chunk-pair. Head leads each
            # batch: its chain to the output adds is the long one.
            in_tiles = []
            for b in range(BPC):
                ht_ = loads.tile([P, DC, S], BF16, tag="head")
                dt_ = loads.tile([P, DC, S], F8, tag="dep")
                in_tiles.append((ht_, dt_))
            # w/bias on the (empty) scalar ring: drains before Q1 traffic
            # starts, so its completion sem fires early.
            wbt = singles.tile([P, 16], F32)
            nc.scalar.dma_start(out=wbt, in_=wb)
            # first head pair as two single-chunk DMAs -> earliest first sem
            nc.sync.dma_start(out=in_tiles[0][0][:, 0:1], in_=head[0, :, 0:1])
            nc.sync.dma_start(out=in_tiles[0][0][:, 1:2], in_=head[0, :, 1:2])
            for b in range(BPC):
                ht_, dt_ = in_tiles[b]
                for pr in range(DC // 2):
                    if b == 0 and pr == 0:
                        continue
                    nc.sync.dma_start(
                        out=ht_[:, 2 * pr : 2 * pr + 2], in_=head[b, :, 2 * pr : 2 * pr + 2]
                    )
                for pr in range(DC // 2):
                    if b == BPC - 1 and pr == DC // 2 - 1:
                        # last dep pair as two chunks -> earlier final sem
                        nc.sync.dma_start(
                            out=dt_[:, 2 * pr : 2 * pr + 1], in_=dep[b, :, 2 * pr : 2 * pr + 1]
                        )
                        nc.sync.dma_start(
                            out=dt_[:, 2 * pr + 1 : 2 * pr + 2],
                            in_=dep[b, :, 2 * pr + 1 : 2 * pr + 2],
                        )
                    else:
                        nc.sync.dma_start(
                            out=dt_[:, 2 * pr : 2 * pr + 2], in_=dep[b, :, 2 * pr : 2 * pr + 2]
                        )

            # ---- engine warmup during the load latency ----
            warm_sb = singles.tile([P, 256], BF16)
            nc.vector.memset(warm_sb, 1.0)
            ones11b = singles.tile([1, 1], BF16)
            nc.vector.memset(ones11b, 1.0)
            warm_act = singles.tile([1, 1], F32)
            nc.scalar.copy(out=warm_act, in_=ones11b)  # triggers ACT table load
            ps_warm = psum_warm.tile([P, 256], F32)
            for i in range(N_WARM):
                nc.tensor.matmul(
                    ps_warm,
                    lhsT=warm_sb[:, :1].broadcast_to((P, P)),
                    rhs=warm_sb,
                    start=True,
                    stop=True,
                )
            wct = singles.tile([P, 2 * DC], BF16)
            nc.vector.tensor_copy(wct, wbt[:, : 2 * DC])
            bt = wbt[:, 2 * DC : 2 * DC + 1]  # f32 bias column, used as AP

            for b in range(BPC):
                ht_, dt_ = in_tiles[b]

                # s_h row [1, S] (head leads the stream)
                ps_shr = psum_shr.tile([1, S], F32, tag="shr")
                for h in range(2):
                    for dc in range(DC):
                        nc.tensor.matmul(
                            ps_shr[:, h * HALF : (h + 1) * HALF],
                            lhsT=wct[:, DC + dc : DC + dc + 1],
                            rhs=ht_[:, dc, h * HALF : (h + 1) * HALF],
                            start=(dc == 0),
                            stop=(dc == DC - 1),
                        )
                shr_sb = svec.tile([1, S], BF16, tag="shr_sb")
                nc.scalar.copy(out=shr_sb, in_=ps_shr)

                # s_d broadcast into PSUM [128, S] (accumulate over d-chunks)
                ps_sdb = psum_sdb.tile([P, S], F32, tag="sdb")
                for h in range(2):
                    for dc in range(DC):
                        nc.tensor.matmul(
                            ps_sdb[:, h * HALF : (h + 1) * HALF],
                            lhsT=wct[:, dc : dc + 1].broadcast_to((P, P)),
                            rhs=dt_[:, dc, h * HALF : (h + 1) * HALF],
                            start=(dc == 0),
                            stop=(dc == DC - 1),
                        )
                # DVE takes the h0 half (its matmul group retires ~1 us before
                # h1's, so this copy overlaps the h1 matmuls); ACT takes h1.
                sdb_sb = bcast.tile([P, S], BF16, tag="sdb_sb")
                nc.vector.tensor_scalar_add(sdb_sb[:, :HALF], ps_sdb[:, :HALF], bt)
                nc.scalar.add(out=sdb_sb[:, HALF:], in_=ps_sdb[:, HALF:], add=bt)

                # transpose s_h row into per-partition columns [128, 8]
                ps_shc = psum_shc.tile([P, RC], F32, tag="shc")
                for c in range(RC):
                    nc.tensor.matmul(
                        ps_shc[:, c : c + 1],
                        lhsT=shr_sb[:, c * P : (c + 1) * P],
                        rhs=ones11b,
                        start=True,
                        stop=True,
                    )
                shc = svec.tile([P, RC], F32, tag="shc_sb")
                nc.vector.tensor_copy(shc, ps_shc)

                # outputs: chunk c rows get sdb_sb + s_h[c*128+p]. b0 stores
                # all ride the sync ring (ACT dispatching them would block
                # its b1 copies behind the store waits); b1 pairs split
                # across both rings to parallelize the final dispatches.
                last = b == BPC - 1
                for t in range(NPAIR):
                    ot = outs.tile([P, 2, S], BF16, tag="ot")
                    for i in range(2):
                        c = 2 * t + i
                        nc.vector.tensor_scalar_add(
                            ot[:, i, :], sdb_sb, shc[:, c : c + 1]
                        )
                    eng = nc.scalar if (last and t % 2 == 0) else nc.sync
                    eng.dma_start(out=out[b, t], in_=ot)
    nc.compile()
    return nc


def _prep_input(x: np.ndarray, dtype) -> np.ndarray:
    """[B, S, D] f32 -> [B, P, DC, S] with [b, p, c, j] = x[b, j, c*P+p]."""
    xt = x.astype(dtype).transpose(0, 2, 1)  # [B, D, S] view
    xt = xt.reshape(B, DC, P, S)  # forces the copy
    return xt.swapaxes(1, 2)  # [B, P, DC, S] view


def kernel(head, dep, edge_W, edge_b, _trace=False):
    nc = build_program()

    head_t = _prep_input(head, NP_BF16)
    dep_t = _prep_input(dep, NP_F8)
    # wb[p, i] = w_d[i*128+p] (i<6) | w_h[(i-6)*128+p] (6<=i<12) | b | pad
    wb = np.zeros((P, 16), dtype=np.float32)
    wb[:, :DC] = edge_W[0, D:].reshape(DC, P).T
    wb[:, DC : 2 * DC] = edge_W[0, :D].reshape(DC, P).T
    wb[:, 2 * DC] = edge_b[0]

    in_maps = []
    for k in range(N_CORES):
        in_maps.append(
            {
                "head": np.ascontiguousarray(head_t[k * BPC : (k + 1) * BPC]),
                "dep": np.ascontiguousarray(dep_t[k * BPC : (k + 1) * BPC]),
                "wb": wb,
            }
        )
    res = run_bass_kernel_spmd(nc, in_maps, core_ids=list(range(N_CORES)), trace=_trace)
    raw = np.concatenate([r["out"] for r in res.results], axis=0)  # [B,4,P,2,S] bf16
    out = (
        raw.transpose(0, 1, 3, 2, 4).reshape(B, S, S).astype(np.float32)
    )
    if _trace:
        return out, res
    return out


if __name__ == "__main__":
    rng = np.random.default_rng(0)
    head = rng.standard_normal((B, S, D), dtype=np.float32)
    dep = rng.standard_normal((B, S, D), dtype=np.float32)
    edge_W = rng.standard_normal((1, 2 * D), dtype=np.float32)
    edge_b = rng.standard_normal((1,), dtype=np.float32)
    out = kernel(head, dep, edge_W, edge_b)
    ref = (
        head @ edge_W[0, :D]
    )[:, :, None] + (dep @ edge_W[0, D:])[:, None, :] + edge_b[0]
    err = np.abs(out - ref).max() / np.abs(ref).max()
    print("max rel err:", err)
